# revision 30
# baseline (speedup 1.0000x reference)
"""DDiT block kernel for 8 Trainium2 NeuronCores.

Sharding: data-parallel over (batch, sequence-half) -> 8 shards. Each core
processes one batch's full sequence through LN1/K/V (needed for attention),
but only its 512 query tokens through Q/attention/MLP.

Device layout is feature-major (model dim on partitions, tokens on the free
axis), which makes every adaLN modulation a per-partition scalar and lets all
matmuls consume activations without transposes. Host folds the adaLN scale
and LN weight into the qkv/mlp1 weights, and the shift-vectors into biases.
Tokens are rotated per-core so queries are always tokens [0:512).

Perf structure: a warmup matmul burst flips the PE HAM clock gate before real
work; the rope rotate-half swap runs on the PE (permutation matmul) instead
of SBUF-SBUF DMAs; reciprocals run as exp(-ln(x)) on the scalar engine
(table ops) because DVE reciprocal is an 8-cycle/elem iterative op; the
K-projection, rope, scores, exp and attn@V for each head pair are emitted
together so the softmax exp stream overlaps the projection matmul stream.

Matmuls run in bf16 with fp32 PSUM accumulation; LN stats and residuals
stay fp32.
"""

import numpy as np
import ml_dtypes

BF = ml_dtypes.bfloat16

B, S, D, H, HD = 4, 1024, 1024, 16, 64
Q = 512          # queries per core
KO = 8           # 1024 dim / 128 partitions
MLP = 4096
LN_EPS = 1e-5
WARMUP_MM = 72   # 64-wide warmup matmuls to trip the HAM clock gate

_CACHE = {}


# ----------------------------------------------------------------------------
# host-side layout helpers
# ----------------------------------------------------------------------------

def _pieces(W, m_piece):
    """[K, M] weight -> [n_pieces, 128, K//128, m_piece] bf16, contiguous."""
    K, M = W.shape
    ko = K // 128
    Wr = np.asarray(W, np.float32).reshape(ko, 128, M).transpose(1, 0, 2)
    n = M // m_piece
    out = Wr.reshape(128, ko, n, m_piece).transpose(2, 0, 1, 3)
    return np.ascontiguousarray(out.astype(BF))


def _pvec(v):
    """[M] vector -> [128, M//128] f32 (partition-major chunks)."""
    v = np.asarray(v, np.float32)
    return np.ascontiguousarray(v.reshape(-1, 128).T)


def _sel_matrix():
    """[8, 8*64] bf16: column block h selects row h broadcast to 64 rows."""
    Sl = np.zeros((8, 8 * 64), np.float32)
    for h in range(8):
        Sl[h, h * 64:(h + 1) * 64] = 1.0
    return np.ascontiguousarray(Sl.astype(BF))


def _swap_matrix():
    """[128,128] permutation: swap 32-row halves within each 64-row block."""
    P = np.zeros((128, 128), np.float32)
    for g in range(2):
        r = g * 64
        for i in range(32):
            P[r + 32 + i, r + i] = 1.0
            P[r + i, r + 32 + i] = 1.0
    return np.ascontiguousarray(P.astype(BF))


# ----------------------------------------------------------------------------
# device program
# ----------------------------------------------------------------------------

def _build_program(repeat=1):
    import concourse.bass as bass
    import concourse.mybir as mybir
    import concourse.tile as tile
    from concourse import bacc

    f32 = mybir.dt.float32
    bf = mybir.dt.bfloat16
    AF = mybir.ActivationFunctionType
    ALU = mybir.AluOpType
    ts = bass.ts

    nc = bacc.Bacc("TRN2", target_bir_lowering=False, debug=False,
                   enable_asserts=False)

    def din(name, shape, dt=bf):
        return nc.dram_tensor(name, shape, dt, kind="ExternalInput").ap()

    xb_d = din("xb", [D, S])                      # bf16 x, feature-major
    cs_d = din("cs", [128, 2 * S])                # cos | sign*sin rows
    msd_d = din("msd", [128, 2 * S])              # LN1 mu | rstd, host-computed
    vec_d = din("vecs", [128, 80], f32)           # packed bias/gate vectors
    swp_d = din("swp", [128, 128])                # rope half-swap permutation
    sel_d = din("sel", [8, 8 * 64])               # head-select broadcast matrices
    wqkv_d = din("wqkv", [6, 128, KO, 512])
    wo_d = din("wao", [2, 128, KO, 512])
    w1_d = din("wm1", [8, 128, KO, 512])
    w2_d = din("wm2", [8, 128, 32, 128])
    yt_d = nc.dram_tensor("yt", [D, Q], f32, kind="ExternalOutput").ap()

    with tile.TileContext(nc) as tc:
        with tc.tile_pool(name="sb", bufs=1) as sb, \
             tc.tile_pool(name="ps", bufs=1, space="PSUM") as ps:
            for _rep in range(repeat):

                def psum():
                    return ps.tile([128, 512], f32, tag="p", bufs=4, name="pt")

                def psum2():
                    return ps.tile([128, 1024], f32, tag="p2", bufs=2, name="pt2")

                # ---- constants + warmup ----
                ones_b = sb.tile([128, 128], bf, tag="ones", bufs=1)
                nc.vector.memset(ones_b[:], 1.0)
                eps_ap = sb.tile([128, 1], f32, tag="eps", bufs=1)
                nc.vector.memset(eps_ap[:], LN_EPS)

                def rstd_from_var(dst, var_ap):
                    """dst(bf16) = 1/sqrt(var+eps): ACT sqrt + DVE reciprocal."""
                    sd = sb.tile([128, 512], f32, tag="tmpf", bufs=3, name="sd")
                    nc.scalar.activation(sd[:], var_ap, AF.Sqrt, bias=eps_ap[:])
                    with nc.allow_low_precision(reason="bf16 LN rstd"):
                        nc.vector.reciprocal(dst, sd[:])

                wps = psum()
                for _w in range(WARMUP_MM):
                    nc.tensor.matmul(wps[:, 0:64], ones_b[:], ones_b[:, 0:64],
                                     start=True, stop=True)
                wdump = sb.tile([128, 64], bf, tag="wdump", bufs=1)
                nc.vector.tensor_copy(wdump[:], wps[:, 0:64])

                # ---- input DMAs ----
                xb16 = sb.tile([128, KO, S], bf, tag="biga", bufs=1, name="xb16")
                xbr = xb_d.rearrange("(ko p) t -> p ko t", p=128)
                for ko in range(KO):
                    if ko % 2 == 0:
                        nc.sync.dma_start(xb16[:, ko, :], xbr[:, ko, :])
                    else:
                        nc.scalar.dma_start(xb16[:, ko, :], xbr[:, ko, :])
                cssb = sb.tile([128, 2 * S], bf, tag="cs", bufs=1)
                nc.sync.dma_start(cssb[:], cs_d[:])
                cc = cssb[:, 0:S]
                ss = cssb[:, S:2 * S]
                msd = sb.tile([128, 2 * S], bf, tag="msd", bufs=1)
                nc.scalar.dma_start(msd[:], msd_d[:])
                vecs = sb.tile([128, 80], f32, tag="vecs", bufs=1)
                nc.sync.dma_start(vecs[:], vec_d[:])
                swp = sb.tile([128, 128], bf, tag="swp", bufs=1)
                nc.sync.dma_start(swp[:], swp_d[:])
                sel16 = sb.tile([8, 8 * 64], bf, tag="sel", bufs=1)
                nc.sync.dma_start(sel16[:], sel_d[:])
                dmat2 = [sb.tile([8, 512], bf, tag="dmat", bufs=2, name="dmat")
                         for _ in range(2)]
                rcp2 = [sb.tile([8, 512], bf, tag="rcp16", bufs=2, name="rcp")
                        for _ in range(2)]

                def wpiece(dram, i, shape):
                    t = sb.tile(shape, bf, tag="w8", bufs=6, name="w")
                    nc.scalar.dma_start(t[:], dram[i])
                    return t

                wq_sb = [wpiece(wqkv_d, i, [128, KO, 512]) for i in range(2)]
                wv_sb = [wpiece(wqkv_d, 4 + i, [128, KO, 512]) for i in range(2)]
                wk_sb = [wpiece(wqkv_d, 2 + i, [128, KO, 512]) for i in range(2)]

                # ---- LN1: host-computed mu/rstd (broadcast rows in msd) ----
                mu01 = msd[:, 0:S]
                rstd01 = msd[:, S:2 * S]
                g16 = []
                for ko in range(KO):
                    g = sb.tile([128, S], bf, tag="g16", bufs=8, name="g16")
                    for tb in range(2):
                        tm = sb.tile([128, 512], bf, tag="qslab", bufs=4, name="tm")
                        nc.vector.tensor_tensor(tm[:], xb16[:, ko, ts(tb, 512)],
                                                mu01[:, ts(tb, 512)], ALU.subtract)
                        nc.vector.tensor_tensor(g[:, ts(tb, 512)], tm[:],
                                                rstd01[:, ts(tb, 512)], ALU.mult)
                    g16.append(g)

                # ---- Q projection + rope (512 query tokens) ----
                qr8 = []
                for jo in range(KO):
                    pq = psum()
                    for ko in range(KO):
                        nc.tensor.matmul(pq[:], wq_sb[jo // 4][:, ko, ts(jo % 4, 128)],
                                         g16[ko][:, 0:Q], start=(ko == 0),
                                         stop=(ko == KO - 1))
                    qa = sb.tile([128, Q], bf, tag="qslab", bufs=4, name="qa")
                    nc.scalar.add(qa[:], pq[:], vecs[:, jo:jo + 1])
                    psw = psum()
                    nc.tensor.matmul(psw[:], swp[:], qa[:], start=True, stop=True)
                    qsw = sb.tile([128, Q], bf, tag="qslab", bufs=4, name="qsw")
                    nc.scalar.copy(qsw[:], psw[:])
                    t1 = sb.tile([128, Q], bf, tag="qslab", bufs=4, name="qt1")
                    nc.vector.tensor_tensor(t1[:], qa[:], cc[:, 0:Q], ALU.mult)
                    t2 = sb.tile([128, Q], bf, tag="qslab", bufs=4, name="qt2")
                    nc.vector.tensor_tensor(t2[:], qsw[:], ss[:, 0:Q], ALU.mult)
                    qr = sb.tile([128, Q], bf, tag="act1k", bufs=16, name="qr")
                    nc.vector.tensor_tensor(qr[:], t1[:], t2[:], ALU.add)
                    qr8.append(qr)

                # ---- V projection, token-major, ones-column per head ----
                v_sb = sb.tile([128, KO, H, 66], bf, tag="vsb", bufs=1, name="vsb")
                nc.vector.memset(v_sb[:, :, :, 64:65], 1.0)
                for to in range(KO):
                    for nb in range(2):
                        pv = psum()
                        for ko in range(KO):
                            nc.tensor.matmul(pv[:], g16[ko][:, ts(to, 128)],
                                             wv_sb[nb][:, ko, :],
                                             start=(ko == 0), stop=(ko == KO - 1))
                        nc.scalar.copy(
                            v_sb[:, to, nb * 8:(nb + 1) * 8, 0:64],
                            pv[:].rearrange("p (h d) -> p h d", d=64))

                wo_sb = [wpiece(wo_d, i, [128, KO, 512]) for i in range(2)]

                # ---- fused K projection + rope + attention, per head pair ----
                oT8 = [sb.tile([128, Q], bf, tag="act1k", bufs=16, name="oT")
                       for _ in range(KO)]
                ou16 = []
                for jo in range(KO):
                    # K projection for this head pair (all 1024 tokens)
                    ka = sb.tile([128, S], bf, tag="kslab", bufs=4, name="ka")
                    kr = sb.tile([128, S], bf, tag="krt", bufs=2, name="kr")
                    for tb in range(2):
                        pk = psum()
                        for ko in range(KO):
                            nc.tensor.matmul(pk[:],
                                             wk_sb[jo // 4][:, ko, ts(jo % 4, 128)],
                                             g16[ko][:, ts(tb, 512)],
                                             start=(ko == 0), stop=(ko == KO - 1))
                        nc.vector.tensor_scalar_add(ka[:, ts(tb, 512)], pk[:],
                                                    vecs[:, 8 + jo:9 + jo])
                        psw2 = psum()
                        nc.tensor.matmul(psw2[:], swp[:], ka[:, ts(tb, 512)],
                                         start=True, stop=True)
                        t1k = sb.tile([128, 512], bf, tag="qslab", bufs=4, name="kt1")
                        nc.vector.tensor_tensor(t1k[:], ka[:, ts(tb, 512)],
                                                cc[:, ts(tb, 512)], ALU.mult)
                        t2k = sb.tile([128, 512], bf, tag="qslab", bufs=4, name="kt2")
                        nc.vector.tensor_tensor(t2k[:], psw2[:], ss[:, ts(tb, 512)],
                                                ALU.mult)
                        nc.vector.tensor_tensor(kr[:, ts(tb, 512)], t1k[:], t2k[:],
                                                ALU.add)

                    # scores + exp, scoresT layout [keys, q]
                    probs = {0: [], 1: []}
                    for half in range(4):
                        pbig = {}
                        for sub in range(2):
                            r0 = sub * 64
                            big = psum2()
                            for kk in range(2):
                                kt = half * 2 + kk
                                nc.tensor.matmul(big[:, ts(kk, 512)],
                                                 kr[r0:r0 + 64, ts(kt, 128)],
                                                 qr8[jo][r0:r0 + 64, :],
                                                 start=True, stop=True,
                                                 tile_position=(r0, 0))
                            pbig[sub] = big
                        for sub in range(2):
                            pb = sb.tile([128, 1024], bf, tag="probs", bufs=5, name="pb")
                            nc.scalar.activation(pb[:], pbig[sub][:], AF.Exp,
                                                 scale=0.125)
                            probs[sub].append(pb)

                    # attn @ V; stash unnormalized o + denominator row
                    for sub in range(2):
                        h = 2 * jo + sub
                        po = psum()
                        for kt in range(KO):
                            nc.tensor.matmul(po[0:65, :], v_sb[:, kt, h, 0:65],
                                             probs[sub][kt // 2][:, ts(kt % 2, 512)],
                                             start=(kt == 0), stop=(kt == KO - 1))
                        ou = sb.tile([65, 512], bf, tag="ou", bufs=17, name="ou")
                        nc.vector.tensor_copy(ou[:], po[0:65, :])
                        ou16.append(ou)
                        hw = h % 8
                        if h % 2 == 0:
                            nc.sync.dma_start(dmat2[h // 8][hw:hw + 1, :],
                                              ou[64:65, :])
                        else:
                            nc.scalar.dma_start(dmat2[h // 8][hw:hw + 1, :],
                                                ou[64:65, :])

                    # wave normalize: wave 0 after pair 5 (overlaps pairs 6-7),
                    # wave 1 after the loop
                    if jo == 5 or jo == KO - 1:
                        for w in ([0] if jo == 5 else [1]):
                            w0 = w * 8
                            with nc.allow_low_precision(reason="bf16 softmax denom"):
                                nc.vector.reciprocal(rcp2[w][:], dmat2[w][:])
                            for h in range(w0, w0 + 8):
                                rb = psum()
                                nc.tensor.matmul(rb[0:64, :],
                                                 sel16[:, ts(h % 8, 64)],
                                                 rcp2[w][:], start=True, stop=True)
                                o16 = sb.tile([64, 512], bf, tag="o16", bufs=4,
                                              name="o16")
                                nc.vector.tensor_tensor(o16[:], ou16[h][0:64, :],
                                                        rb[0:64, :], ALU.mult)
                                r0 = (h % 2) * 64
                                if h % 2 == 0:
                                    nc.sync.dma_start(oT8[h // 2][r0:r0 + 64, :],
                                                      o16[:])
                                else:
                                    nc.scalar.dma_start(oT8[h // 2][r0:r0 + 64, :],
                                                        o16[:])

                w1_sb = [wpiece(w1_d, i, [128, KO, 512]) for i in range(8)]

                # ---- attn out + gated residual (x2 kept bf16) ----
                x2 = []
                for do in range(KO):
                    py = psum()
                    for ko in range(KO):
                        nc.tensor.matmul(py[:], wo_sb[do // 4][:, ko, ts(do % 4, 128)],
                                         oT8[ko][:], start=(ko == 0),
                                         stop=(ko == KO - 1))
                    t = sb.tile([128, 512], f32, tag="tmpf", bufs=3, name="tao")
                    nc.scalar.activation(t[:], py[:], AF.Identity,
                                         bias=vecs[:, 16 + do:17 + do],
                                         scale=vecs[:, 24 + do:25 + do])
                    xx = sb.tile([128, Q], bf, tag="act1k", bufs=16, name="x2")
                    nc.vector.tensor_tensor(xx[:], t[:], xb16[:, do, 0:Q], ALU.add)
                    x2.append(xx)

                # ---- LN2 (512 tokens) ----
                p1 = psum()
                p2 = psum()
                for ko in range(KO):
                    sq2 = sb.tile([128, Q], bf, tag="qslab", bufs=4, name="sq2")
                    nc.scalar.square(sq2[:], x2[ko][:])
                    nc.tensor.matmul(p1[:], ones_b[:], x2[ko][:], start=(ko == 0),
                                     stop=(ko == KO - 1))
                    nc.tensor.matmul(p2[:], ones_b[:], sq2[:], start=(ko == 0),
                                     stop=(ko == KO - 1))
                mu2 = sb.tile([128, 512], bf, tag="stats4", bufs=2, name="mu2")
                nc.vector.tensor_scalar_mul(mu2[:], p1[:], 1.0 / D)
                ex2b = sb.tile([128, 512], f32, tag="tmpf", bufs=3, name="ex2b")
                nc.vector.tensor_scalar_mul(ex2b[:], p2[:], 1.0 / D)
                var2 = sb.tile([128, 512], f32, tag="tmpf", bufs=3, name="var2")
                nc.vector.tensor_tensor(var2[:], mu2[:], mu2[:], ALU.mult)
                nc.vector.tensor_tensor(var2[:], ex2b[:], var2[:], ALU.subtract)
                rstd2 = sb.tile([128, 512], bf, tag="stats4", bufs=2, name="rstd2")
                rstd_from_var(rstd2[:], var2[:])
                g2 = []
                for ko in range(KO):
                    tm2 = sb.tile([128, Q], bf, tag="qslab", bufs=4, name="tm2")
                    nc.vector.tensor_tensor(tm2[:], x2[ko][:], mu2[:], ALU.subtract)
                    gk = sb.tile([128, Q], bf, tag="act1k", bufs=16, name="g2")
                    nc.vector.tensor_tensor(gk[:], tm2[:], rstd2[:], ALU.mult)
                    g2.append(gk)

                w2_sb = [wpiece(w2_d, i, [128, 32, 128]) for i in range(8)]

                # ---- MLP ----
                m16 = sb.tile([128, 32, Q], bf, tag="biga", bufs=1, name="m16")
                for mo in range(32):
                    pm = psum()
                    for ko in range(KO):
                        nc.tensor.matmul(pm[:], w1_sb[mo // 4][:, ko, ts(mo % 4, 128)],
                                         g2[ko][:], start=(ko == 0),
                                         stop=(ko == KO - 1))
                    nc.scalar.activation(m16[:, mo], pm[:], AF.Gelu_apprx_tanh,
                                         bias=vecs[:, 32 + mo:33 + mo], scale=1.0)
                yt_r = yt_d.rearrange("(ko p) t -> p ko t", p=128)
                for do in range(KO):
                    pz = psum()
                    for ko in range(32):
                        nc.tensor.matmul(pz[:], w2_sb[do][:, ko, :],
                                         m16[:, ko, :], start=(ko == 0), stop=(ko == 31))
                    t = sb.tile([128, 512], f32, tag="tmpf", bufs=3, name="tz")
                    nc.scalar.activation(t[:], pz[:], AF.Identity,
                                         bias=vecs[:, 64 + do:65 + do],
                                         scale=vecs[:, 72 + do:73 + do])
                    yo = sb.tile([128, 512], f32, tag="yout", bufs=4, name="yo")
                    nc.vector.tensor_tensor(yo[:], t[:], x2[do][:], ALU.add)
                    nc.sync.dma_start(yt_r[:, do, :], yo[:])

    nc.compile()
    return nc


# ----------------------------------------------------------------------------
# host wrapper
# ----------------------------------------------------------------------------

def _prep_shared(inputs):
    x = np.asarray(inputs["x"], np.float32)
    c = np.asarray(inputs["c"], np.float32)
    w_ada = np.asarray(inputs["w_ada"], np.float32)
    b_ada = np.asarray(inputs["b_ada"], np.float32)
    w_qkv = np.asarray(inputs["w_qkv"], np.float32)
    w_ao = np.asarray(inputs["w_attn_out"], np.float32)
    w_m1 = np.asarray(inputs["w_mlp1"], np.float32)
    w_m2 = np.asarray(inputs["w_mlp2"], np.float32)

    mod = c @ w_ada + b_ada
    sh_msa, sc_msa, g_msa, sh_mlp, sc_mlp, g_mlp = np.split(mod, 6, axis=1)
    ln1 = np.asarray(inputs["w_ln1"], np.float32) * (1.0 + sc_msa)   # [4, D]
    ln2 = np.asarray(inputs["w_ln2"], np.float32) * (1.0 + sc_mlp)

    shared = {}
    for b in range(B):
        Wq = w_qkv[:, :D] * ln1[b][:, None]
        Wk = w_qkv[:, D:2 * D] * ln1[b][:, None]
        Wv = w_qkv[:, 2 * D:] * ln1[b][:, None]
        bqkv = sh_msa[b] @ w_qkv
        W1 = w_m1 * ln2[b][:, None]
        bm1 = sh_mlp[b] @ w_m1 + np.asarray(inputs["b_mlp1"], np.float32)
        # packed vector tile [128, 80]:
        # bq 0:8 | bk 8:16 | bao 16:24 | gmsa 24:32 | bm1 32:64 | bm2g 64:72 | gmlp 72:80
        vecs = np.concatenate([
            _pvec(bqkv[:D]), _pvec(bqkv[D:2 * D]),
            _pvec((bqkv[2 * D:] @ w_ao) * g_msa[b]),
            _pvec(g_msa[b]), _pvec(bm1),
            _pvec(np.asarray(inputs["b_mlp2"], np.float32) * g_mlp[b]),
            _pvec(g_mlp[b]),
        ], axis=1)
        wqkv = np.concatenate([_pieces(Wq, 512), _pieces(Wk, 512),
                               _pieces(Wv, 512)], axis=0)
        shared[b] = dict(
            wqkv=np.ascontiguousarray(wqkv), wm1=_pieces(W1, 512),
            vecs=np.ascontiguousarray(vecs.astype(np.float32)),
        )
    wao_p = _pieces(w_ao, 512)
    wm2_p = _pieces(w_m2, 128)
    cos = np.asarray(inputs["cos"], np.float32)
    sin = np.asarray(inputs["sin"], np.float32)
    return shared, wao_p, wm2_p, x, cos, sin


def _make_in_maps(inputs):
    shared, wao_p, wm2_p, x, cos, sin = _prep_shared(inputs)
    swp = _swap_matrix()
    sel = _sel_matrix()
    in_maps = []
    for core in range(8):
        b, half = core // 2, core % 2
        qlo = half * Q
        order = np.concatenate([np.arange(qlo, qlo + Q), np.arange(0, qlo),
                                np.arange(qlo + Q, S)])
        xo = x[b][order]
        mu = xo.mean(axis=1)
        rstd = 1.0 / np.sqrt(xo.var(axis=1) + LN_EPS)
        msd = np.concatenate([np.tile(mu[None, :].astype(BF), (128, 1)),
                              np.tile(rstd[None, :].astype(BF), (128, 1))], axis=1)
        xT = xo.T
        cosT = cos[order].T                      # [32, S]
        sinT = sin[order].T
        cc = np.concatenate([cosT] * 4, 0)
        ssn = np.concatenate([-sinT, sinT, -sinT, sinT], 0)
        cs = np.concatenate([cc, ssn], axis=1).astype(BF)   # [128, 2S]
        sh = shared[b]
        in_maps.append({
            "xb": np.ascontiguousarray(xT.astype(BF)),
            "cs": np.ascontiguousarray(cs),
            "vecs": sh["vecs"],
            "swp": swp, "sel": sel, "msd": np.ascontiguousarray(msd),
            "wqkv": sh["wqkv"], "wao": wao_p, "wm1": sh["wm1"], "wm2": wm2_p,
        })
    return in_maps


def kernel(**inputs):
    from concourse import bass_utils

    if "nc" not in _CACHE:
        _CACHE["nc"] = _build_program()
    nc = _CACHE["nc"]

    in_maps = _make_in_maps(inputs)
    res = bass_utils.run_bass_kernel_spmd(nc, in_maps, core_ids=list(range(8)))

    y = np.zeros((B, S, D), np.float32)
    for core in range(8):
        b, half = core // 2, core % 2
        qlo = half * Q
        y[b, qlo:qlo + Q] = res.results[core]["yt"].T
    return y
